# revision 1
# baseline (speedup 1.0000x reference)
"""GCNGuard 2-layer GNN kernel for 8 Trainium2 NeuronCores (Bass/Tile).

Sharding: edges sorted by (row, col) are split into 8 row-aligned shards;
each core owns a contiguous destination-row range and all its incoming edges.

Per core: destinations are permuted (degree-sorted) into blocks of 128.
Each dest's edges are split by Z-table half (int16 gather index limit: the
node table has >32768 rows) and laid out as padded CSR slot columns per
block. Node features and linear outputs for ALL nodes live in a Z table
([xn_f32 | h_f32] rows) built distributedly and AllGathered. Per-edge data
is fetched by 4-queue-parallel gpsimd dma_gather (descriptor-rate bound).

Per-edge cosine sims: fused tensor_tensor_reduce (DVE). Per-dest stats
(L1 row-normalization, degree, self-loop weight): free-dim reduces in the
CSR layout. Aggregation: ACT per-slot scaling by edge weight + TensorE
identity-matmul accumulation in PSUM.
"""

import os
import numpy as np

import concourse.bass as bass
import concourse.bacc as bacc
import concourse.mybir as mybir
import concourse.tile as tile
from concourse.bass_utils import run_bass_kernel_spmd
from concourse.masks import make_identity
from concourse._compat import cdiv

F32 = mybir.dt.float32
I16 = mybir.dt.int16

N_CORES = 8
LAST_EXEC_NS = None
D_IN = 128
D_HID = 128
D_OUT = 64
SIM_THRESH = 0.1
ZROW_SPLIT = 32768
SLOT_CHUNK = 8          # CSR slots per dma_gather call (8*128 = 1024 idxs)
NQ = 4                  # SWDGE queues (parallel gather descriptor-gen)

AluOp = mybir.AluOpType
ActFn = mybir.ActivationFunctionType


# ---------------------------------------------------------------- host prep

def _wrap_idx_calls(entries):
    """entries [n_calls, 1024] int16 -> wrapped [128, n_calls*64]:
    idx i of a call sits at partition i%16, free i//16, replicated 8x."""
    n_calls = entries.shape[0]
    w = entries.reshape(n_calls, 64, 16).transpose(0, 2, 1)   # [c, 16, 64]
    w = np.concatenate(list(w), axis=1) if n_calls else np.zeros((16, 0), np.int16)
    return np.tile(w, (8, 1)).astype(np.int16)


class _P:
    pass


def build_plan(edge_index, n_node):
    row = np.asarray(edge_index[0], dtype=np.int64)
    col = np.asarray(edge_index[1], dtype=np.int64)
    E = row.shape[0]
    order = np.lexsort((col, row))
    row = row[order]
    col = col[order]

    bnd = [0]
    for k in range(1, N_CORES):
        pos = (k * E) // N_CORES
        bnd.append(int(row[pos]))
    bnd.append(n_node)
    bnd = np.asarray(bnd, dtype=np.int64)
    for k in range(1, N_CORES + 1):
        if bnd[k] <= bnd[k - 1]:
            bnd[k] = bnd[k - 1] + 1
    bnd[-1] = max(bnd[-1], n_node)

    n_dest = bnd[1:] - bnd[:-1]
    nblk_per_core = [cdiv(int(n), 128) for n in n_dest]
    NBLK = max(nblk_per_core)
    S = NBLK * 128                      # Z stripe rows per core

    shard_of = np.searchsorted(bnd, col, side="right") - 1
    zcol = shard_of * S + (col - bnd[shard_of])

    plans = []
    for c in range(N_CORES):
        p = _P()
        p.r0, p.r1 = int(bnd[c]), int(min(bnd[c + 1], n_node))
        p.n_dest = p.r1 - p.r0
        m = (row >= p.r0) & (row < p.r1)
        erow = (row[m] - p.r0).astype(np.int64)
        ez = zcol[m]
        elow = ez < ZROW_SPLIT

        n_pad = NBLK * 128
        deg_low = np.bincount(erow[elow], minlength=n_pad)
        deg_high = np.bincount(erow[~elow], minlength=n_pad)
        perm = np.argsort(-(deg_low * 4096 + deg_high), kind="stable")
        rank = np.empty(n_pad, dtype=np.int64)
        rank[perm] = np.arange(n_pad)

        p.perm = perm
        dl_r = deg_low[perm].reshape(NBLK, 128)
        dh_r = deg_high[perm].reshape(NBLK, 128)
        p.w_low = dl_r.max(axis=1).astype(np.int64)
        p.w_high = dh_r.max(axis=1).astype(np.int64)

        def csr_fill(sel, widths, idx_base):
            rk = rank[erow[sel]]
            zz = ez[sel]
            o = np.lexsort((zz, rk))
            rk = rk[o]
            zz = zz[o]
            if rk.shape[0]:
                newd = np.ones(rk.shape[0], dtype=bool)
                newd[1:] = rk[1:] != rk[:-1]
                starts = np.flatnonzero(newd)
                counts = np.diff(np.append(starts, rk.shape[0]))
                slot = np.arange(rk.shape[0]) - np.repeat(starts, counts)
            else:
                slot = np.zeros(0, dtype=np.int64)
            blk = rk // 128
            pp = rk % 128
            base = np.zeros(len(widths) + 1, dtype=np.int64)
            base[1:] = np.cumsum(widths * 128)
            flat = np.zeros(int(base[-1]), dtype=np.int16)
            maskf = np.zeros(int(base[-1]), dtype=np.float32)
            pos = base[blk] + slot * 128 + pp
            flat[pos] = (zz - idx_base).astype(np.int16)
            maskf[pos] = 1.0
            return flat, maskf, base

        t_low, m_low, b_low = csr_fill(elow, p.w_low, 0)
        t_high, m_high, b_high = csr_fill(~elow, p.w_high, ZROW_SPLIT)

        def wrap_all(flat, widths, base):
            calls = []
            for b in range(len(widths)):
                wb = int(widths[b])
                s = 0
                while s < wb:
                    ns = min(SLOT_CHUNK, wb - s)
                    ent = np.zeros(1024, dtype=np.int16)
                    ent[:ns * 128] = flat[int(base[b]) + s * 128:
                                          int(base[b]) + (s + ns) * 128]
                    calls.append(ent)
                    s += ns
            if not calls:
                return np.zeros((128, 0), dtype=np.int16)
            return _wrap_idx_calls(np.stack(calls))

        p.idx_low = wrap_all(t_low, p.w_low, b_low)
        p.idx_high = wrap_all(t_high, p.w_high, b_high)

        def mask_dev(maskflat, widths, base):
            cols = []
            for b in range(len(widths)):
                wb = int(widths[b])
                seg = maskflat[int(base[b]): int(base[b]) + wb * 128]
                cols.append(seg.reshape(wb, 128).T)
            if not cols:
                return np.zeros((128, 0), dtype=np.float32)
            return np.concatenate(cols, axis=1).astype(np.float32)

        p.pm_low = mask_dev(m_low, p.w_low, b_low)
        p.pm_high = mask_dev(m_high, p.w_high, b_high)

        # unpermute idx: natural local i -> rank[i]  (rank < n_pad <= 32768 req)
        assert n_pad <= 32767, "stripe too large for int16 unpermute gather"
        n_calls_up = cdiv(n_pad, 1024)
        flat_ids = np.zeros(n_calls_up * 1024, dtype=np.int64)
        flat_ids[:n_pad] = rank[np.arange(n_pad)]
        p.idx_up = _wrap_idx_calls(
            flat_ids.reshape(n_calls_up, 1024).astype(np.int16))

        plans.append(p)

    meta = _P()
    meta.bnd = bnd
    meta.NBLK = NBLK
    meta.S = S
    # unified per-block widths across cores (single SPMD program)
    meta.w_low = [max(int(p.w_low[b]) for p in plans) for b in range(NBLK)]
    meta.w_high = [max(int(p.w_high[b]) for p in plans) for b in range(NBLK)]
    return plans, meta


def _pad_tables(plans, meta):
    """Pad each core's idx/padmask tables to the unified per-block widths."""
    NBLK = meta.NBLK

    def expand(p, widths_core, widths_uni, idx_tab, pm_tab):
        # idx_tab cols: per block, ceil(w/8) calls * 64 cols
        # pm_tab cols: per block, w cols
        idx_out = []
        pm_out = []
        io = 0
        mo = 0
        for b in range(NBLK):
            wc = int(widths_core[b])
            wu = int(widths_uni[b])
            ncc = cdiv(wc, SLOT_CHUNK)
            ncu = cdiv(wu, SLOT_CHUNK)
            blkidx = np.zeros((128, ncu * 64), dtype=np.int16)
            blkidx[:, :ncc * 64] = idx_tab[:, io:io + ncc * 64]
            idx_out.append(blkidx)
            blkpm = np.zeros((128, wu), dtype=np.float32)
            blkpm[:, :wc] = pm_tab[:, mo:mo + wc]
            pm_out.append(blkpm)
            io += ncc * 64
            mo += wc
        idx_cat = np.concatenate(idx_out, axis=1) if idx_out else np.zeros((128, 0), np.int16)
        pm_cat = np.concatenate(pm_out, axis=1) if pm_out else np.zeros((128, 0), np.float32)
        return idx_cat, pm_cat

    for p in plans:
        p.idx_low_u, p.pm_low_u = expand(p, p.w_low, meta.w_low, p.idx_low, p.pm_low)
        p.idx_high_u, p.pm_high_u = expand(p, p.w_high, meta.w_high, p.idx_high, p.pm_high)


# ---------------------------------------------------------------- device

def build_nc(meta):
    STAGES = os.environ.get("GCN_STAGES", "s1,cc,s3,L1,s5,L2").split(",")
    NBLK = meta.NBLK
    S = meta.S
    ZROWS = N_CORES * S
    w_low = meta.w_low
    w_high = meta.w_high
    il_cols = sum(cdiv(w, SLOT_CHUNK) for w in w_low) * 64
    ih_cols = sum(cdiv(w, SLOT_CHUNK) for w in w_high) * 64
    ml_cols = sum(w_low)
    mh_cols = sum(w_high)
    up_calls = cdiv(S, 1024)

    nc = bacc.Bacc("TRN2", target_bir_lowering=False, num_swdge_queues=NQ)

    x_own = nc.dram_tensor("x_own", [S, D_IN], F32, kind="ExternalInput")
    x_perm = nc.dram_tensor("x_perm", [S, D_IN], F32, kind="ExternalInput")
    w1 = nc.dram_tensor("w1", [D_IN, D_HID], F32, kind="ExternalInput")
    b1r = nc.dram_tensor("b1r", [128, D_HID], F32, kind="ExternalInput")
    w2 = nc.dram_tensor("w2", [D_HID, D_OUT], F32, kind="ExternalInput")
    b2r = nc.dram_tensor("b2r", [128, D_OUT], F32, kind="ExternalInput")
    idx_low = nc.dram_tensor("idx_low", [128, max(il_cols, 64)], I16, kind="ExternalInput")
    idx_high = nc.dram_tensor("idx_high", [128, max(ih_cols, 64)], I16, kind="ExternalInput")
    idx_up = nc.dram_tensor("idx_up", [128, up_calls * 64], I16, kind="ExternalInput")
    pm_low = nc.dram_tensor("pm_low", [128, max(ml_cols, 1)], F32, kind="ExternalInput")
    pm_high = nc.dram_tensor("pm_high", [128, max(mh_cols, 1)], F32, kind="ExternalInput")
    out = nc.dram_tensor("out", [S, D_OUT], F32, kind="ExternalOutput")

    zin1 = nc.dram_tensor("zin1", [S, 256], F32)
    z1 = nc.dram_tensor("z1", [ZROWS, 256], F32)
    zin2 = nc.dram_tensor("zin2", [S, 192], F32)
    z2 = nc.dram_tensor("z2", [ZROWS, 192], F32)
    hs = nc.dram_tensor("hs", [S, D_HID], F32)

    qn = [0]

    def next_q():
        q = qn[0]
        qn[0] = (qn[0] + 1) % NQ
        return q

    with tile.TileContext(nc) as tc:
        with (
            tc.tile_pool(name="persist", bufs=1) as pers,
            tc.tile_pool(name="work", bufs=4) as pool,
            tc.tile_pool(name="gpool", bufs=4) as gpool,
            tc.tile_pool(name="ipool", bufs=4) as ipool,
            tc.tile_pool(name="psum", bufs=2, space="PSUM") as psum,
            tc.tile_pool(name="apsum", bufs=2, space="PSUM") as apsum,
        ):
            ident = pers.tile([128, 128], F32)
            make_identity(nc, ident[:])
            w1_sb = pers.tile([D_IN, D_HID], F32)
            nc.sync.dma_start(w1_sb[:], w1[:])
            b1_sb = pers.tile([128, D_HID], F32)
            nc.sync.dma_start(b1_sb[:], b1r[:])
            w2_sb = pers.tile([D_HID, D_OUT], F32)
            nc.sync.dma_start(w2_sb[:], w2[:])
            b2_sb = pers.tile([128, D_OUT], F32)
            nc.sync.dma_start(b2_sb[:], b2r[:])

            # persistent per-dest tensors: [128 partitions, NBLK * D] with
            # block b in columns [b*D, (b+1)*D); partition = rank % 128.
            xn_dest = pers.tile([128, NBLK * D_IN], F32)
            h_dest = pers.tile([128, NBLK * D_HID], F32)
            h1_dest = pers.tile([128, NBLK * D_HID], F32)
            h2_dest = pers.tile([128, NBLK * D_OUT], F32)
            out_dest = pers.tile([128, NBLK * D_OUT], F32)

            def lin_norm_tile(xt, w_sb, b_sb, dout):
                """xt [128, 128] SBUF -> (xn [128,128], h [128,dout]) tiles."""
                sq = pool.tile([128, D_IN], F32, tag="lsq")
                n2 = pool.tile([128, 1], F32, tag="ln2")
                nc.vector.tensor_mul(sq[:], xt[:], xt[:])
                nc.vector.tensor_reduce(out=n2[:], in_=sq[:], op=AluOp.add,
                                        axis=mybir.AxisListType.X)
                nrm = pool.tile([128, 1], F32, tag="lnrm")
                nc.scalar.activation(nrm[:], n2[:], ActFn.Sqrt)
                zi = pool.tile([128, 1], F32, tag="lzi")
                nc.vector.tensor_scalar(
                    out=zi[:], in0=nrm[:], scalar1=0.0, scalar2=None,
                    op0=AluOp.is_equal)
                nc.vector.tensor_add(nrm[:], nrm[:], zi[:])
                rn = pool.tile([128, 1], F32, tag="lrn")
                nc.vector.reciprocal(rn[:], nrm[:])
                xnt = pool.tile([128, D_IN], F32, tag="lxn")
                nc.vector.tensor_scalar_mul(xnt[:], xt[:], rn[:])
                xT = psum.tile([128, 128], F32, tag="lxT")
                nc.tensor.transpose(xT[:], xt[:], ident[:])
                xT_sb = pool.tile([128, 128], F32, tag="lxTs")
                nc.vector.tensor_copy(xT_sb[:], xT[:])
                hp = psum.tile([128, dout], F32, tag="lhp")
                nc.tensor.matmul(hp[:], xT_sb[:], w_sb[:], start=True, stop=True)
                ht = pool.tile([128, dout], F32, tag="lht")
                nc.vector.tensor_add(ht[:], hp[:], b_sb[:, :dout])
                return xnt, ht

            # Stage 1: natural stripe -> zin1 ([xn | h1lin])
            for t in range(NBLK):
                xt = pool.tile([128, D_IN], F32, tag="lx")
                nc.sync.dma_start(xt[:], x_own[t * 128:(t + 1) * 128, :])
                xnt, ht = lin_norm_tile(xt, w1_sb, b1_sb, D_HID)
                nc.sync.dma_start(zin1[t * 128:(t + 1) * 128, 0:128], xnt[:])
                nc.sync.dma_start(zin1[t * 128:(t + 1) * 128, 128:256], ht[:])

            # Stage 2: AllGather -> z1
            if "cc" in STAGES:
                nc.gpsimd.collective_compute(
                    "AllGather", AluOp.bypass,
                    replica_groups=[list(range(N_CORES))],
                    ins=[zin1.ap().opt()], outs=[z1.ap().opt()])

            # Stage 3: permuted dest side
            for t in range(NBLK):
                xt = pool.tile([128, D_IN], F32, tag="lx")
                nc.sync.dma_start(xt[:], x_perm[t * 128:(t + 1) * 128, :])
                xnt, ht = lin_norm_tile(xt, w1_sb, b1_sb, D_HID)
                nc.vector.tensor_copy(xn_dest[:, t * D_IN:(t + 1) * D_IN], xnt[:])
                nc.vector.tensor_copy(h_dest[:, t * D_HID:(t + 1) * D_HID], ht[:])

            def edge_layer(z, zw, dpay, xn_d, h_self, acc_out, relu):
                # xn_d/h_self/acc_out: [128, NBLK*stride] col-blocked tiles
                zxl = z[0:ZROW_SPLIT, 0:D_IN]
                zxh = z[ZROW_SPLIT:ZROWS, 0:D_IN]
                zpl = z[0:ZROW_SPLIT, D_IN:D_IN + dpay]
                zph = z[ZROW_SPLIT:ZROWS, D_IN:D_IN + dpay]
                ioff_l = 0
                ioff_h = 0
                moff_l = 0
                moff_h = 0
                for b in range(NBLK):
                    wl, wh = w_low[b], w_high[b]
                    wt = wl + wh
                    if wt == 0:
                        continue
                    ncl = cdiv(wl, SLOT_CHUNK)
                    nch = cdiv(wh, SLOT_CHUNK)
                    # block idx tables into SBUF
                    ib_l = ipool.tile([128, max(ncl * 64, 8)], I16, tag="ibl")
                    if ncl:
                        nc.sync.dma_start(ib_l[:, 0:ncl * 64],
                                          idx_low[:, ioff_l:ioff_l + ncl * 64])
                    ib_h = ipool.tile([128, max(nch * 64, 8)], I16, tag="ibh")
                    if nch:
                        nc.sync.dma_start(ib_h[:, 0:nch * 64],
                                          idx_high[:, ioff_h:ioff_h + nch * 64])
                    pmb = pool.tile([128, wt], F32, tag="pmb")
                    if wl:
                        nc.sync.dma_start(pmb[:, 0:wl], pm_low[:, moff_l:moff_l + wl])
                    if wh:
                        nc.sync.dma_start(pmb[:, wl:wt], pm_high[:, moff_h:moff_h + wh])

                    xnb = xn_d[:, b * D_IN:(b + 1) * D_IN]
                    sim = pool.tile([128, wt], F32, tag="sim")

                    def sweep(zsrc, wcls, ib, col0, payload, ew=None, accp=None,
                              first_mm=False):
                        s = 0
                        ci = 0
                        while s < wcls:
                            ns = min(SLOT_CHUNK, wcls - s)
                            g = gpool.tile([128, SLOT_CHUNK, zsrc.shape[1]], F32,
                                           tag="gp" if payload else "gs")
                            nc.gpsimd.dma_gather(
                                g[:, 0:ns, :], zsrc,
                                ib[:, ci * 64:ci * 64 + ns * 8],
                                ns * 128, ns * 128, zsrc.shape[1],
                                elem_step=zw, queue_num=next_q())
                            for j in range(ns):
                                cidx = col0 + s + j
                                if not payload:
                                    scr = pool.tile([128, D_IN], F32, tag="scr")
                                    nc.vector.tensor_mul(scr[:], g[:, j, :], xnb)
                                    nc.vector.tensor_reduce(
                                        out=sim[:, cidx:cidx + 1], in_=scr[:],
                                        op=AluOp.add, axis=mybir.AxisListType.X)
                                else:
                                    tmp = pool.tile([128, dpay], F32, tag="tmp")
                                    nc.scalar.activation(
                                        tmp[:], g[:, j, :], ActFn.Copy,
                                        scale=ew[:, cidx:cidx + 1])
                                    st = first_mm and (s == 0) and (j == 0)
                                    nc.tensor.matmul(accp[:], ident[:], tmp[:],
                                                     start=st, stop=False)
                            s += ns
                            ci += 1

                    # sim sweeps
                    sweep(zxl, wl, ib_l, 0, payload=False)
                    sweep(zxh, wh, ib_h, wl, payload=False)

                    # stats
                    msk = pool.tile([128, wt], F32, tag="msk")
                    nc.vector.tensor_scalar(
                        out=msk[:], in0=sim[:], scalar1=SIM_THRESH, scalar2=None,
                        op0=AluOp.is_ge)
                    nc.vector.tensor_mul(msk[:], msk[:], pmb[:])
                    simt = pool.tile([128, wt], F32, tag="simt")
                    nc.vector.tensor_mul(simt[:], sim[:], msk[:])
                    rowsum = pool.tile([128, 1], F32, tag="rowsum")
                    nc.vector.tensor_reduce(
                        out=rowsum[:], in_=simt[:], op=AluOp.add,
                        axis=mybir.AxisListType.X)
                    deg = pool.tile([128, 1], F32, tag="deg")
                    nc.vector.tensor_reduce(
                        out=deg[:], in_=msk[:], op=AluOp.add,
                        axis=mybir.AxisListType.X)
                    zi = pool.tile([128, 1], F32, tag="zi")
                    nc.vector.tensor_scalar(
                        out=zi[:], in0=rowsum[:], scalar1=0.0, scalar2=None,
                        op0=AluOp.is_equal)
                    nc.vector.tensor_add(rowsum[:], rowsum[:], zi[:])
                    rr = pool.tile([128, 1], F32, tag="rr")
                    nc.vector.reciprocal(rr[:], rowsum[:])
                    ew = pool.tile([128, wt], F32, tag="ew")
                    nc.scalar.activation(ew[:], simt[:], ActFn.Exp, scale=rr[:])
                    nc.vector.tensor_mul(ew[:], ew[:], msk[:])
                    dp1 = pool.tile([128, 1], F32, tag="dp1")
                    nc.vector.tensor_scalar_add(dp1[:], deg[:], 1.0)
                    lam = pool.tile([128, 1], F32, tag="lam")
                    nc.vector.reciprocal(lam[:], dp1[:])
                    sw = pool.tile([128, 1], F32, tag="sw")
                    nc.scalar.activation(sw[:], lam[:], ActFn.Exp)

                    # aggregation
                    accp = apsum.tile([128, dpay], F32, tag="accp")
                    tmp0 = pool.tile([128, dpay], F32, tag="tmp0")
                    nc.vector.tensor_scalar_mul(
                        tmp0[:], h_self[:, b * dpay:(b + 1) * dpay], sw[:])
                    nc.tensor.matmul(accp[:], ident[:], tmp0[:], start=True, stop=False)
                    sweep(zpl, wl, ib_l, 0, payload=True, ew=ew, accp=accp)
                    sweep(zph, wh, ib_h, wl, payload=True, ew=ew, accp=accp)
                    # close the accumulation group
                    zz = pool.tile([128, dpay], F32, tag="zz")
                    nc.vector.memset(zz[:], 0.0)
                    nc.tensor.matmul(accp[:], ident[:], zz[:], start=False, stop=True)
                    acc = pool.tile([128, dpay], F32, tag="acc")
                    if relu:
                        nc.vector.tensor_scalar_max(acc[:], accp[:], 0.0)
                    else:
                        nc.vector.tensor_copy(acc[:], accp[:])
                    nc.vector.tensor_copy(acc_out[:, b * dpay:(b + 1) * dpay], acc[:])

                    ioff_l += ncl * 64
                    ioff_h += nch * 64
                    moff_l += wl
                    moff_h += wh

            # Layer 1
            if "L1" in STAGES:
                edge_layer(z1, 256, D_HID, xn_dest, h_dest, h1_dest, True)
            else:
                for b in range(NBLK):
                    nc.vector.tensor_copy(
                        h1_dest[:, b * D_HID:(b + 1) * D_HID],
                        h_dest[:, b * D_HID:(b + 1) * D_HID])

            # Stage 5: unpermute h1, build zin2 ([hn1 | h2lin]), dest-side tensors
            for t in range(NBLK):
                nc.sync.dma_start(hs[t * 128:(t + 1) * 128, :],
                                  h1_dest[:, t * D_HID:(t + 1) * D_HID])
            for u in range(up_calls if "s5" in STAGES else 0):
                g = gpool.tile([128, SLOT_CHUNK, D_HID], F32, tag="ug")
                iu = ipool.tile([128, 64], I16, tag="iu")
                nc.sync.dma_start(iu[:], idx_up[:, u * 64:(u + 1) * 64])
                nc.gpsimd.dma_gather(
                    g[:, :, :], hs[:], iu[:], 1024, 1024, D_HID,
                    queue_num=next_q())
                for j in range(SLOT_CHUNK):
                    t = u * SLOT_CHUNK + j
                    if t >= NBLK:
                        break
                    xnt, ht = lin_norm_tile(g[:, j, :], w2_sb, b2_sb, D_OUT)
                    nc.sync.dma_start(zin2[t * 128:(t + 1) * 128, 0:128], xnt[:])
                    nc.sync.dma_start(zin2[t * 128:(t + 1) * 128, 128:192], ht[:])
            if "cc" in STAGES:
                nc.gpsimd.collective_compute(
                    "AllGather", AluOp.bypass,
                    replica_groups=[list(range(N_CORES))],
                    ins=[zin2.ap().opt()], outs=[z2.ap().opt()])
            # dest-side for layer 2: hn1_perm / h2_perm from h1_dest (SBUF)
            for t in range(NBLK):
                xnt, ht = lin_norm_tile(h1_dest[:, t * D_HID:(t + 1) * D_HID],
                                        w2_sb, b2_sb, D_OUT)
                nc.vector.tensor_copy(xn_dest[:, t * D_IN:(t + 1) * D_IN], xnt[:])
                nc.vector.tensor_copy(h2_dest[:, t * D_OUT:(t + 1) * D_OUT], ht[:])

            # Layer 2
            if "L2" in STAGES:
                edge_layer(z2, 192, D_OUT, xn_dest, h2_dest, out_dest, False)
            else:
                for b in range(NBLK):
                    nc.vector.tensor_copy(
                        out_dest[:, b * D_OUT:(b + 1) * D_OUT],
                        h2_dest[:, b * D_OUT:(b + 1) * D_OUT])

            # output
            for t in range(NBLK):
                nc.sync.dma_start(out[t * 128:(t + 1) * 128, :],
                                  out_dest[:, t * D_OUT:(t + 1) * D_OUT])

    nc.compile()
    return nc


# ---------------------------------------------------------------- entry

def kernel(x, edge_index, W1, b1, W2, b2, _debug=None):
    x = np.asarray(x, dtype=np.float32)
    edge_index_np = np.asarray(edge_index)
    W1 = np.asarray(W1, dtype=np.float32)
    b1 = np.asarray(b1, dtype=np.float32)
    W2 = np.asarray(W2, dtype=np.float32)
    b2 = np.asarray(b2, dtype=np.float32)
    n_node = x.shape[0]

    plans, meta = build_plan(edge_index_np, n_node)
    _pad_tables(plans, meta)
    nc = build_nc(meta)

    S = meta.S
    in_maps = []
    for c, p in enumerate(plans):
        xo = np.zeros((S, D_IN), dtype=np.float32)
        xo[:p.n_dest] = x[p.r0:p.r1]
        glob = np.minimum(p.perm + p.r0, n_node - 1)
        valid = (p.perm < p.n_dest)
        xp = x[glob] * valid[:, None]
        in_maps.append({
            "x_own": xo,
            "x_perm": xp.astype(np.float32),
            "w1": W1,
            "b1r": np.tile(b1[None, :], (128, 1)).astype(np.float32),
            "w2": W2,
            "b2r": np.tile(b2[None, :], (128, 1)).astype(np.float32),
            "idx_low": _fit(p.idx_low_u, nc_shape(nc, "idx_low")),
            "idx_high": _fit(p.idx_high_u, nc_shape(nc, "idx_high")),
            "idx_up": _fit(p.idx_up, nc_shape(nc, "idx_up")),
            "pm_low": _fit(p.pm_low_u, nc_shape(nc, "pm_low")),
            "pm_high": _fit(p.pm_high_u, nc_shape(nc, "pm_high")),
        })

    if _debug and _debug.get("sim"):
        from concourse.bass_interp import MultiCoreSim
        sim = MultiCoreSim(nc, N_CORES)
        for c, core in sim.cores.items():
            for k, v in in_maps[c].items():
                core.tensor(k)[:] = v
        sim.simulate()
        outs = [{"out": np.array(sim.cores[c].mem_tensor("out"))}
                for c in range(N_CORES)]
    else:
        res = run_bass_kernel_spmd(nc, in_maps, list(range(N_CORES)),
                                   **(_debug or {}))
        outs = res.results
        global LAST_EXEC_NS
        LAST_EXEC_NS = res.exec_time_ns

    out_full = np.zeros((n_node, D_OUT), dtype=np.float32)
    for c, p in enumerate(plans):
        o = outs[c]["out"]          # [S, 64] permuted
        inv = p.perm                # rank -> local dest
        for_rank = o[:len(inv)]
        loc = np.zeros((meta.NBLK * 128, D_OUT), dtype=np.float32)
        loc[inv] = for_rank
        out_full[p.r0:p.r1] = loc[:p.n_dest]
    return out_full


def nc_shape(nc, name):
    for alloc in nc.m.functions[0].allocations:
        if isinstance(alloc, mybir.MemoryLocationSet) and alloc.memorylocations[0].name == name:
            return tuple(alloc.tensor_shape)
    raise KeyError(name)


def _fit(arr, shape):
    outa = np.zeros(shape, dtype=arr.dtype)
    sl = tuple(slice(0, min(a, b)) for a, b in zip(arr.shape, shape))
    outa[sl] = arr[sl]
    return outa

